# revision 27
# baseline (speedup 1.0000x reference)
"""Meet-in-the-middle DP: pure-DVE loop with direct image reads, built in
manual Block mode (explicit semaphores, no TileContext).

Algorithm:
- Host pre-packs each core's shard as [P=128 partitions, 64 sbuf-rows, 128]
  f16 where sbuf-row 2r is original row r (fwd chain) and sbuf-row 2r+1 is
  original row 63-r (bwd chain), the two per-partition samples adjacent.
  Every DP step's image row is a contiguous 2D [128,128] slice (bwd read
  with stride -1), consumed by the scans directly as data1.
- Seeds are memsets: m0 = [BIAS, BIG.., 0, BIG..] makes iteration 0's scan
  emit the row-0 prefix sums; the -start/2 endpoint correction and the
  seam join (min over down/diag candidates of zf+zb) run on the host,
  which receives the final zF/zB vectors (516B/partition).
- Loop = 4 DVE ops/row-step [sF, sB, mF', mB'] at the model's 642ns floor
  (scan 194ns, no DVE perf mode for tensor_tensor_scan; min 127ns, 2x_1p;
  min/scan cannot leave DVE on this ISA). Every RAW pair has >=127ns of
  engine-time separation, beyond the ~60ns SBUF write-ack window, so
  same-engine program order is sufficient and the loop carries no sems.
- Input DMA: a 3-row first chunk starts the loop at the DMA first-byte
  floor (691 issue + 625 HWDGE + 650 DGE + 273 transfer + 900 DMA-sem
  + 7ns inline-wait resolution = 3146ns); later chunks grow geometrically
  so every completion sem lands ahead of its first consuming scan.
- Output: one DMACopy of the merged [P, 2, 129] f16 state tile, gated by
  the last scan's then_inc; a final SP wait on the DMA-completion sem
  keeps the program end honest.

Why Block mode (vs TileContext, worth ~385ns): wait_ge here inlines into
the next op's sync info (+7ns after a DMA sem) where tile mode emits an
EventSemaphore gate on the sequencer (+106ns), and the epilogue has one
fewer barrier round (~286ns).

Packing guard: slot0 carries +BIAS (via m0[0]=BIAS) so the w128 scan carry
cannot leak sample0 -> sample1; the bwd chain reverses slots+columns, so
each sample's seam sum carries exactly one +BIAS (subtracted host-side).
"""

import sys

import numpy as np

sys.path.insert(0, "/opt/trn_rl_repo")

import concourse.bacc as bacc
import concourse.mybir as mybir
from concourse.bass_utils import run_bass_kernel_spmd

P = 128
Q = 2
H = 64
W = 64
QW = Q * W
STEPS = 32         # F rows 0..31, B rows 63..32
NB_CORE = P * Q
N_CORES = 8
BIG = 3.0e4
BIAS = 16.0
F16 = mybir.dt.float16
MIN = mybir.AluOpType.min
ADD = mybir.AluOpType.add

CHUNKS = [(0, 3), (3, 8), (8, 18), (18, 56), (56, 64)]

_CACHE = {}


def _build():
    nc = bacc.Bacc("TRN2", debug=False, target_bir_lowering=False,
                   num_devices=N_CORES)
    img_d = nc.dram_tensor("images", [P, H, QW], F16, kind="ExternalInput")
    out_d = nc.dram_tensor("out", [P, 2, QW + 1], F16, kind="ExternalOutput")

    # io threshold (16 per completed DMA) gating each scan's image row
    need = {}
    for k, (a, b) in enumerate(CHUNKS, start=1):
        for row in range(a, b):
            need[("F" if row % 2 == 0 else "B", row // 2)] = 16 * k

    with (nc.Block() as block,
          nc.sbuf_tensor("imgT", [P, H, QW], F16) as imgT,
          nc.sbuf_tensor("zfb", [P, 2, QW + 1], F16) as zfb,
          nc.sbuf_tensor("mF", [P, QW], F16) as mF_t,
          nc.sbuf_tensor("mB", [P, QW], F16) as mB_t,
          nc.semaphore("io") as io,
          nc.semaphore("dv") as dv):

        @block.sync
        def _(sync):
            for a, b in CHUNKS:
                sync.dma_start(imgT[:, a:b, :], img_d[:, a:b, :]).then_inc(io, 16)
            sync.wait_ge(dv, 1)
            sync.dma_start(out_d[:], zfb[:]).then_inc(io, 16)
            # hold program end until the output DMA's completion sem fires,
            # then reset the sems so a re-execution of the loaded program
            # starts clean (manual mode has no tile-context sem-clear round)
            sync.wait_ge(io, 16 * (len(CHUNKS) + 1))
            sync.sem_clear(io)
            sync.sem_clear(dv)

        @block.vector
        def _(dve):
            m = {"F": mF_t, "B": mB_t}
            zi = {"F": 0, "B": 1}
            for d in "FB":
                dve.memset(zfb[:, zi[d], 0:1], BIG)
                dve.memset(m[d][:], BIG)
                dve.memset(m[d][:, 0:1], BIAS)
                dve.memset(m[d][:, W:W + 1], 0.0)

            state = {"th": 0}

            def gate(d, r):
                th = need[(d, r)]
                if th > state["th"]:
                    dve.wait_ge(io, th)   # inlines into the next op
                    state["th"] = th

            def sstep(d, r):
                row = imgT[:, 2 * r, :] if d == "F" else imgT[:, 2 * r + 1, ::-1]
                return dve.tensor_tensor_scan(
                    out=zfb[:, zi[d], 1:], data0=m[d][:], data1=row,
                    initial=BIG, op0=MIN, op1=ADD)

            def mstep(d):
                dve.tensor_tensor(out=m[d][:], in0=zfb[:, zi[d], 1:],
                                  in1=zfb[:, zi[d], 0:QW], op=MIN)

            last = None
            for r in range(STEPS):
                gate("F", r)
                sstep("F", r)
                gate("B", r)
                last = sstep("B", r)
                if r + 1 < STEPS:
                    mstep("F")
                    mstep("B")
            last.then_inc(dv, 1)

    nc.compile()
    return nc


def get_nc():
    if "nc" not in _CACHE:
        _CACHE["nc"] = _build()
    return _CACHE["nc"]


_ROW_ORD = np.empty(H, dtype=np.int64)
_ROW_ORD[0::2] = np.arange(H // 2)
_ROW_ORD[1::2] = H - 1 - np.arange(H // 2)


def kernel(images: np.ndarray, **run_kwargs) -> np.ndarray:
    B = images.shape[0]
    assert images.shape == (B, H, W) and B == N_CORES * NB_CORE
    images = np.ascontiguousarray(images, dtype=np.float32)
    img16 = images.astype(np.float16)
    in_maps = []
    for c in range(N_CORES):
        shard = img16[c * NB_CORE:(c + 1) * NB_CORE]
        s = shard.reshape(Q, P, H, W).transpose(1, 2, 0, 3)[:, _ROW_ORD]
        in_maps.append({"images": np.ascontiguousarray(s).reshape(P, H, QW)})
    nc = get_nc()
    res = run_bass_kernel_spmd(nc, in_maps, core_ids=list(range(N_CORES)),
                               **run_kwargs)
    out = np.empty((B,), dtype=np.float32)
    for c in range(N_CORES):
        zz = res.results[c]["out"].astype(np.float32)
        zf = zz[:, 0, 1:].reshape(P, Q, W)
        zb = zz[:, 1, 1:].reshape(P, Q, W)[:, ::-1, ::-1]
        cand = zf + zb
        np.minimum(cand[:, :, :W - 1], zf[:, :, :W - 1] + zb[:, :, 1:],
                   out=cand[:, :, :W - 1])
        v = cand.min(axis=2) - BIAS
        out[c * NB_CORE:(c + 1) * NB_CORE] = v.T.reshape(-1)
    out -= 0.5 * (images[:, 0, 0] + images[:, H - 1, W - 1])
    if run_kwargs:
        return out, res
    return out


# revision 28
# speedup vs baseline: 1.0595x; 1.0595x over previous
"""Meet-in-the-middle DP: pure-DVE loop with direct image reads, built in
manual Block mode (explicit semaphores, no TileContext).

Algorithm:
- Host pre-packs each core's shard as [P=128 partitions, 64 sbuf-rows, 128]
  f16 where sbuf-row 2r is original row r (fwd chain) and sbuf-row 2r+1 is
  original row 63-r (bwd chain), the two per-partition samples adjacent.
  Every DP step's image row is a contiguous 2D [128,128] slice (bwd read
  with stride -1), consumed by the scans directly as data1.
- Seeds are memsets: m0 = [BIAS, BIG.., 0, BIG..] makes iteration 0's scan
  emit the row-0 prefix sums; the -start/2 endpoint correction and the
  seam join (min over down/diag candidates of zf+zb) run on the host,
  which receives the final zF/zB vectors (516B/partition).
- Loop = 4 DVE ops/row-step [sF, sB, mF', mB'] at the model's 642ns floor
  (scan 194ns, no DVE perf mode for tensor_tensor_scan; min 127ns, 2x_1p;
  min/scan cannot leave DVE on this ISA). Every RAW pair has >=127ns of
  engine-time separation, beyond the ~60ns SBUF write-ack window, so
  same-engine program order is sufficient and the loop carries no sems.
- Input DMA: a 3-row first chunk starts the loop at the DMA first-byte
  floor (691 issue + 625 HWDGE + 650 DGE + 273 transfer + 900 DMA-sem
  + 7ns inline-wait resolution = 3146ns); later chunks grow geometrically
  so every completion sem lands ahead of its first consuming scan.
- Output: one DMACopy of the merged [P, 2, 129] f16 state tile, gated by
  the last scan's then_inc; a final SP wait on the DMA-completion sem
  keeps the program end honest.

Why Block mode (vs TileContext, worth ~385ns): wait_ge here inlines into
the next op's sync info (+7ns after a DMA sem) where tile mode emits an
EventSemaphore gate on the sequencer (+106ns), and the epilogue has one
fewer barrier round (~286ns).

Packing guard: slot0 carries +BIAS (via m0[0]=BIAS) so the w128 scan carry
cannot leak sample0 -> sample1; the bwd chain reverses slots+columns, so
each sample's seam sum carries exactly one +BIAS (subtracted host-side).
"""

import sys

import numpy as np

sys.path.insert(0, "/opt/trn_rl_repo")

import concourse.bacc as bacc
import concourse.mybir as mybir
from concourse.bass_utils import run_bass_kernel_spmd

P = 128
Q = 2
H = 64
W = 64
QW = Q * W
STEPS = 32         # F rows 0..31, B rows 63..32
NB_CORE = P * Q
N_CORES = 8
BIG = 3.0e4
BIAS = 16.0
F16 = mybir.dt.float16
MIN = mybir.AluOpType.min
ADD = mybir.AluOpType.add

CHUNKS = [(0, 3), (3, 8), (8, 18), (18, 56), (56, 64)]

_CACHE = {}


def _build():
    nc = bacc.Bacc("TRN2", debug=False, target_bir_lowering=False,
                   num_devices=N_CORES)
    img_d = nc.dram_tensor("images", [P, H, QW], F16, kind="ExternalInput")
    out_d = nc.dram_tensor("out", [P, 2, QW + 1], F16, kind="ExternalOutput")

    # io threshold (16 per completed DMA) gating each scan's image row
    need = {}
    for k, (a, b) in enumerate(CHUNKS, start=1):
        for row in range(a, b):
            need[("F" if row % 2 == 0 else "B", row // 2)] = 16 * k

    with (nc.Block() as block,
          nc.sbuf_tensor("imgT", [P, H, QW], F16) as imgT,
          nc.sbuf_tensor("zfb", [P, 2, QW + 1], F16) as zfb,
          nc.sbuf_tensor("mF", [P, QW], F16) as mF_t,
          nc.sbuf_tensor("mB", [P, QW], F16) as mB_t,
          nc.semaphore("io") as io,
          nc.semaphore("dv") as dv):

        @block.scalar
        def _(act):
            # reset leftover sem values from a prior execution of the loaded
            # program (manual mode has no tile-context sem-clear round). On
            # the idle ACT engine this finishes by ~250ns, well before the
            # first increment (~3.1us) or DVE's first wait (~840ns), and
            # costs nothing on the critical path.
            act.sem_clear(io)
            act.sem_clear(dv)

        @block.sync
        def _(sync):
            for a, b in CHUNKS:
                sync.dma_start(imgT[:, a:b, :], img_d[:, a:b, :]).then_inc(io, 16)
            sync.wait_ge(dv, 1)
            sync.dma_start(out_d[:], zfb[:]).then_inc(io, 16)
            # hold program end until the output DMA's completion sem fires
            sync.wait_ge(io, 16 * (len(CHUNKS) + 1))

        @block.vector
        def _(dve):
            m = {"F": mF_t, "B": mB_t}
            zi = {"F": 0, "B": 1}
            for d in "FB":
                dve.memset(zfb[:, zi[d], 0:1], BIG)
                dve.memset(m[d][:], BIG)
                dve.memset(m[d][:, 0:1], BIAS)
                dve.memset(m[d][:, W:W + 1], 0.0)

            state = {"th": 0}

            def gate(d, r):
                th = need[(d, r)]
                if th > state["th"]:
                    dve.wait_ge(io, th)   # inlines into the next op
                    state["th"] = th

            def sstep(d, r):
                row = imgT[:, 2 * r, :] if d == "F" else imgT[:, 2 * r + 1, ::-1]
                return dve.tensor_tensor_scan(
                    out=zfb[:, zi[d], 1:], data0=m[d][:], data1=row,
                    initial=BIG, op0=MIN, op1=ADD)

            def mstep(d):
                dve.tensor_tensor(out=m[d][:], in0=zfb[:, zi[d], 1:],
                                  in1=zfb[:, zi[d], 0:QW], op=MIN)

            last = None
            for r in range(STEPS):
                gate("F", r)
                sstep("F", r)
                gate("B", r)
                last = sstep("B", r)
                if r + 1 < STEPS:
                    mstep("F")
                    mstep("B")
            last.then_inc(dv, 1)

    nc.compile()
    return nc


def get_nc():
    if "nc" not in _CACHE:
        _CACHE["nc"] = _build()
    return _CACHE["nc"]


_ROW_ORD = np.empty(H, dtype=np.int64)
_ROW_ORD[0::2] = np.arange(H // 2)
_ROW_ORD[1::2] = H - 1 - np.arange(H // 2)


def kernel(images: np.ndarray, **run_kwargs) -> np.ndarray:
    B = images.shape[0]
    assert images.shape == (B, H, W) and B == N_CORES * NB_CORE
    images = np.ascontiguousarray(images, dtype=np.float32)
    img16 = images.astype(np.float16)
    in_maps = []
    for c in range(N_CORES):
        shard = img16[c * NB_CORE:(c + 1) * NB_CORE]
        s = shard.reshape(Q, P, H, W).transpose(1, 2, 0, 3)[:, _ROW_ORD]
        in_maps.append({"images": np.ascontiguousarray(s).reshape(P, H, QW)})
    nc = get_nc()
    res = run_bass_kernel_spmd(nc, in_maps, core_ids=list(range(N_CORES)),
                               **run_kwargs)
    out = np.empty((B,), dtype=np.float32)
    for c in range(N_CORES):
        zz = res.results[c]["out"].astype(np.float32)
        zf = zz[:, 0, 1:].reshape(P, Q, W)
        zb = zz[:, 1, 1:].reshape(P, Q, W)[:, ::-1, ::-1]
        cand = zf + zb
        np.minimum(cand[:, :, :W - 1], zf[:, :, :W - 1] + zb[:, :, 1:],
                   out=cand[:, :, :W - 1])
        v = cand.min(axis=2) - BIAS
        out[c * NB_CORE:(c + 1) * NB_CORE] = v.T.reshape(-1)
    out -= 0.5 * (images[:, 0, 0] + images[:, H - 1, W - 1])
    if run_kwargs:
        return out, res
    return out
